# revision 33
# baseline (speedup 1.0000x reference)
"""Multi-head self-attention (B=2, L=2048, D=1024, H=16, causal) on 8
Trainium2 NeuronCores.

Sharding: tensor-parallel over heads x data-parallel over batch.
Core c (0..7) handles batch b = c//4 and heads 4*(c%4) .. 4*(c%4)+3.
Each core computes partial = (softmax(qk^T/8) @ v_heads) @ Wo[:, cols]^T of
shape [L, D]; the host sums the 4 partials of each batch group.

Per-core kernel design (validated on device at rel_err ~4e-3):
  - inputs stream in as bf16 (halves the serial DMA prefix); DMA lands
    directly in matmul-ready tiles, no staging copies; q/k stay f32r
    after the fp32-accumulated projection
  - the q/k j=0 projection wave is emitted contraction-chunk-major so
    it tracks the chunk-interleaved input DMAs; zero-weight `warm`
    matmuls (start=False adds of 0) keep the PE clock ramp at full
    speed through the DMA-gated stretches
  - scores are computed TRANSPOSED (S^T = k q^T per 128-row key chunk,
    causal columns only); exp runs on ScalarE PSUM->SBUF producing P^T
    in bf16 with the 1/sqrt(dh) scale folded in; the diagonal block is
    masked AFTER exp by a 0/1 lower-tri multiply on the Pool engine
    (SBUF-only: GPSIMD/Pool cannot touch PSUM on trn2)
  - P@V is flipped: each 128-query output tile O[q, d] accumulates over
    key chunks with P^T slices as the stationary operand and the 65-wide
    (64 dims + ones column) bf16 v block as the moving operand - the PE
    streams 65 columns per chunk instead of the full query range
  - the ones column gives per-row softmax sums; normalization is a DVE
    reciprocal + per-partition tensor_scalar multiply (no broadcast
    matmuls, no extra activations)
  - normalized O tiles are transposed on the PE (bf16 identity matmul)
    into the packed [128, 2, L] layout the output projection consumes
    with full K=128 contraction chunks
  - one flat software-pipelined loop runs all 4 heads' (scores -> pv ->
    normalize -> transpose) stages at fixed lags; the v projection rides
    head 0, the j=1 q/k chains ride heads 1-2 just-in-time, and the
    output projection + bf16 store DMAs ride head 3, so PE and ScalarE
    stay co-busy end to end
"""

import numpy as np

B, L, D, H = 2, 2048, 1024, 16
DH = D // H  # 64
N_CORES = 8
HEADS_PER_CORE = 4
HD = HEADS_PER_CORE * DH  # 256 head dims per core
NK = D // 128  # 8 contraction chunks
LT = L // 128  # 16 L tiles

_CACHE = {}


# ---------------------------------------------------------------------------
# walrus compat: this compiler build accepts at most ONE sync-wait command
# per instruction, while TileContext attaches one wait per producer proc.
# Hoist surplus waits onto same-engine NOPs inserted just before the
# offending instruction (identical AND semantics).
# ---------------------------------------------------------------------------
def _split_waits(nc):
    import bass_rust
    import concourse.mybir as mybir

    for fn in nc.m.functions:
        for bb in fn.blocks:
            insts = list(bb.instructions)
            out = []
            changed = False
            for inst in insts:
                si = inst.sync_info
                waits = list(si.on_wait) if si is not None and si.on_wait else []
                if len(waits) > 1:
                    changed = True
                    for w in waits[:-1]:
                        out.append(
                            mybir.InstNoOp(
                                name=nc.get_next_instruction_name(),
                                engine=inst.engine,
                                bass_nofuse=True,
                                sync_info=bass_rust.SyncInfo(
                                    on_wait=[w], on_update=[]
                                ),
                            )
                        )
                    inst.sync_info = bass_rust.SyncInfo(
                        on_wait=[waits[-1]], on_update=list(si.on_update or [])
                    )
                out.append(inst)
            if changed:
                try:
                    bb.instructions = out
                except Exception:
                    bb.instructions.clear()
                    bb.instructions.extend(out)


def _build_program():
    import concourse.bass as bass
    import concourse.mybir as mybir
    import concourse.tile as tile

    f32 = mybir.dt.float32
    f32r = mybir.dt.float32r
    bf16 = mybir.dt.bfloat16
    AF = mybir.ActivationFunctionType

    nc = bass.Bass("TRN2", target_bir_lowering=False, debug=False)
    xT_d = nc.dram_tensor("xT", [D, L], bf16, kind="ExternalInput")
    wq_d = nc.dram_tensor("wqT", [D, HD], bf16, kind="ExternalInput")
    wk_d = nc.dram_tensor("wkT", [D, HD], bf16, kind="ExternalInput")
    wv_d = nc.dram_tensor("wvT", [D, HD], bf16, kind="ExternalInput")
    wo_d = nc.dram_tensor("woT", [HD, D], bf16, kind="ExternalInput")
    tm_d = nc.dram_tensor("trimask", [128, 128], bf16, kind="ExternalInput")
    id_d = nc.dram_tensor("ident", [128, 128], bf16, kind="ExternalInput")
    out_d = nc.dram_tensor("out", [L, D], bf16, kind="ExternalOutput")

    with tile.TileContext(nc, pool_alloc_mode="queue") as tc:
        with tc.tile_pool(name="persist", bufs=1) as persist:
            qTr = [persist.tile([128, L], f32r, name=f"qTr{j}") for j in range(2)]
            kTr = [persist.tile([128, L], f32r, name=f"kTr{j}") for j in range(2)]
            # v with a ones column per head: [key%128, keychunk, 65*h + u]
            v_sb = persist.tile([128, LT, HEADS_PER_CORE * (DH + 1)], bf16)
            tm_t = persist.tile([128, 128], bf16)
            id_t = persist.tile([128, 128], bf16)
            woTr = persist.tile([128, 2, D], bf16)
            # packed O^T: [:, j, q] partitions 0-63 head 2j, 64-127 head 2j+1
            otP = persist.tile([128, 2, L], bf16)
            xTr = [
                persist.tile([128, L], bf16, name=f"xTr{c}") for c in range(NK)
            ]
            wq_sb = persist.tile([128, NK, HD], bf16)
            wk_sb = persist.tile([128, NK, HD], bf16)
            wv_sb = persist.tile([128, NK, HD], bf16)

            # warm-up fodder: zero stationary operand makes a matmul that
            # adds 0 to any psum region - used to hold the PE clock ramp
            # at full speed through the DMA-gated load phase
            zt = persist.tile([128, 128], bf16)
            nc.gpsimd.memset(zt[:], 0.0)

            # ---- input DMA, one ordered queue, chunk-interleaved so the
            # q/k wave can follow the transfers chunk by chunk
            nc.sync.dma_start(tm_t[:], tm_d[:])
            nc.sync.dma_start(id_t[:], id_d[:])
            wqv = wq_d[:].rearrange("(c p) n -> p c n", p=128)
            wkv = wk_d[:].rearrange("(c p) n -> p c n", p=128)
            wvv = wv_d[:].rearrange("(c p) n -> p c n", p=128)
            for c in range(NK):
                nc.sync.dma_start(wq_sb[:, c, :], wqv[:, c, :])
                nc.sync.dma_start(wk_sb[:, c, :], wkv[:, c, :])
                nc.sync.dma_start(xTr[c][:], xT_d[c * 128 : (c + 1) * 128, :])
            nc.sync.dma_start(
                wv_sb[:], wv_d[:].rearrange("(c p) n -> p c n", p=128)
            )
            nc.sync.dma_start(
                woTr[:], wo_d[:].rearrange("(j p) n -> p j n", p=128)
            )

            # ones column of v (memset once over the strided view)
            vview = v_sb[:].rearrange("p t (h u) -> p t h u", u=DH + 1)
            nc.gpsimd.memset(vview[:, :, :, DH], 1.0)

            def warm(ps, n):
                for _ in range(n):
                    nc.tensor.matmul(
                        ps[:, 0:128], zt[:], id_t[:],
                        start=False, stop=False, skip_group_check=True,
                    )

            # ---------------- phase A wave 1: q,k for head pair j=0 ------
            # (j=1 and v chains are interleaved into the attention loops)
            with tc.tile_pool(name="psA", bufs=8, space="PSUM") as psA:
                wave = []
                for g in range(4):
                    for wt_sb, dst in ((wq_sb, qTr), (wk_sb, kTr)):
                        ps = psA.tile([128, 512], f32, tag="pa",
                                      name=f"pa{len(wave)}")
                        wave.append((ps, wt_sb, dst, g))
                warm(wave[0][0], 8)
                for c in range(NK):
                    for ps, wt_sb, dst, g in wave:
                        nc.tensor.matmul(
                            ps[:],
                            wt_sb[:, c, 0:128],
                            xTr[c][:, g * 512 : (g + 1) * 512],
                            start=(c == 0),
                            stop=(c == NK - 1),
                        )
                    if c < NK - 1:
                        warm(wave[0][0], 2)
                for i, (ps, wt_sb, dst, g) in enumerate(wave):
                    # only the evacs exp(0,0) depends on go to ACT; the
                    # rest queue on DVE so the first exp starts ASAP
                    eng = nc.scalar.copy if i < 2 else nc.vector.tensor_copy
                    eng(dst[0][:, g * 512 : (g + 1) * 512], ps[:])

            # ------------- phase B: attention, software-pipelined -------
            with (
                tc.tile_pool(name="ptp", bufs=2) as ptp,
                tc.tile_pool(name="onp", bufs=3) as onp,
                tc.tile_pool(name="rcp", bufs=3) as rcp,
                tc.tile_pool(name="stg", bufs=3) as stg,
                tc.tile_pool(name="psS", bufs=2, space="PSUM") as psS,
                tc.tile_pool(name="psO", bufs=2, space="PSUM") as psO,
                tc.tile_pool(name="psT", bufs=1, space="PSUM") as psT,
                tc.tile_pool(name="psX", bufs=1, space="PSUM") as psX,
            ):
                PT = {}
                o_ps = {}
                o_nrm = {}

                def hparams(h):
                    return h // 2, 64 * (h % 2)

                def qk_chain(pool, wt_sb, dst, j, g, tag, evac):
                    ps = pool.tile([128, 512], f32, tag=tag, name="qkc")
                    for c in range(NK):
                        nc.tensor.matmul(
                            ps[:],
                            wt_sb[:, c, j * 128 : (j + 1) * 128],
                            xTr[c][:, g * 512 : (g + 1) * 512],
                            start=(c == 0),
                            stop=(c == NK - 1),
                        )
                    if evac == "scalar":
                        nc.scalar.copy(dst[j][:, g * 512 : (g + 1) * 512], ps[:])
                    else:
                        nc.vector.tensor_copy(
                            dst[j][:, g * 512 : (g + 1) * 512], ps[:]
                        )

                def v_chain(pool, t, tag):
                    ps = pool.tile([128, 512], f32, tag=tag, name="vc")
                    for c in range(NK):
                        nc.tensor.matmul(
                            ps[:, 0:HD],
                            xTr[c][:, t * 128 : (t + 1) * 128],
                            wv_sb[:, c, :],
                            start=(c == 0),
                            stop=(c == NK - 1),
                        )
                    vdst = v_sb[:, t, :].rearrange("p (h u) -> p h u", u=DH + 1)
                    nc.vector.tensor_copy(
                        vdst[:, :, 0:DH],
                        ps[:, 0:HD].rearrange("p (h u) -> p h u", u=DH),
                    )

                def scores(h, m):
                    hp, ho = hparams(h)
                    c0 = 128 * m
                    w = L - c0
                    PT[h, m] = ptp.tile(
                        [128, w], bf16, tag=f"pt{m}", name=f"pt{h}_{m}"
                    )
                    # the overlapped head-tail steps borrow the psX bank so
                    # the next head's first scores own the psS rotation
                    use_px = h in (0, 2) and m >= 14
                    sw = min(1024, w)
                    ps = (
                        psX.tile([128, 512], f32, tag="px", name="spx")
                        if use_px
                        else psS.tile([128, 1024], f32, tag="st", name="sps")
                    )
                    for n0 in range(0, sw, 512):
                        nw = min(512, sw - n0)
                        nc.tensor.matmul(
                            ps[:, n0 : n0 + nw],
                            kTr[hp][ho : ho + 64, c0 : c0 + 128],
                            qTr[hp][
                                ho : ho + 64,
                                c0 + n0 : c0 + n0 + nw,
                            ],
                            start=True,
                            stop=True,
                        )
                    nc.scalar.activation(
                        PT[h, m][:, 0:sw],
                        ps[:, 0:sw],
                        AF.Exp,
                        scale=0.125,
                    )
                    # zero the masked (key > q) part of the diagonal
                    # block after exp - off the PE->ACT critical path
                    nc.gpsimd.tensor_mul(
                        PT[h, m][:, 0:128], PT[h, m][:, 0:128], tm_t[:]
                    )

                def scores2(h, m):
                    # far half (cols 1024+) of the score row, one step
                    # later: doubles the psS tile reuse distance. PV chains
                    # never need these columns for their first 8 chunks, so
                    # the pv lag stays 1.
                    hp, ho = hparams(h)
                    c0 = 128 * m
                    w = L - c0
                    if w <= 1024:
                        return
                    sw = w - 1024
                    ps = psS.tile([128, 1024], f32, tag="st", name="sps2")
                    for n0 in range(0, sw, 512):
                        nw = min(512, sw - n0)
                        nc.tensor.matmul(
                            ps[:, n0 : n0 + nw],
                            kTr[hp][ho : ho + 64, c0 : c0 + 128],
                            qTr[hp][
                                ho : ho + 64,
                                c0 + 1024 + n0 : c0 + 1024 + n0 + nw,
                            ],
                            start=True,
                            stop=True,
                        )
                    nc.scalar.activation(
                        PT[h, m][:, 1024 : 1024 + sw],
                        ps[:, 0:sw],
                        AF.Exp,
                        scale=0.125,
                    )

                def pv_chain(h, t):
                    o_ps[h, t] = psO.tile(
                        [128, DH + 1], f32, tag="o", name=f"ops{h}_{t}"
                    )
                    for mm in range(t + 1):
                        nc.tensor.matmul(
                            o_ps[h, t][:],
                            PT[h, mm][:, (t - mm) * 128 : (t - mm) * 128 + 128],
                            v_sb[:, mm, h * (DH + 1) : (h + 1) * (DH + 1)],
                            start=(mm == 0),
                            stop=(mm == t),
                        )

                def normalize(h, t):
                    r = rcp.tile([128, 1], f32, tag="r", name="rc")
                    nc.vector.reciprocal(r[:], o_ps[h, t][:, DH : DH + 1])
                    o_nrm[h, t] = onp.tile(
                        [128, DH], bf16, tag="on", name=f"onrm{h}_{t}"
                    )
                    nc.vector.tensor_scalar_mul(
                        o_nrm[h, t][:], o_ps[h, t][:, 0:DH], r[:]
                    )
                    del o_ps[h, t]

                def transpose(h, t):
                    hp, ho = hparams(h)
                    tp = psT.tile([64, 128], bf16, tag="tp", name="tpp")
                    nc.tensor.transpose(tp[:], o_nrm[h, t][:], id_t[:])
                    nc.vector.tensor_copy(
                        otP[ho : ho + 64, hp, t * 128 : (t + 1) * 128],
                        tp[:],
                    )
                    del o_nrm[h, t]

                def out_proj(t):
                    stage = stg.tile([128, D], bf16, tag="ostage", name="stt")
                    ps0 = psX.tile([128, 512], f32, tag="px", name="cpx")
                    ps1 = psS.tile([128, 1024], f32, tag="st", name="cps")
                    pss = [ps0, ps1[:, 0:512]]
                    for n2 in range(2):
                        for j in range(2):
                            nc.tensor.matmul(
                                pss[n2],
                                otP[:, j, t * 128 : (t + 1) * 128],
                                woTr[:, j, n2 * 512 : (n2 + 1) * 512],
                                start=(j == 0),
                                stop=(j == 1),
                            )
                        if t >= 11:
                            nc.scalar.copy(
                                stage[:, n2 * 512 : (n2 + 1) * 512], pss[n2]
                            )
                        else:
                            nc.vector.tensor_copy(
                                stage[:, n2 * 512 : (n2 + 1) * 512], pss[n2]
                            )
                    nc.sync.dma_start(
                        out_d[t * 128 : (t + 1) * 128, :], stage[:]
                    )

                # head h's 16 scores steps start at BASE[h]; heads 1 and
                # 3 begin two steps before the previous head's scores end,
                # pairing their big early exps with the tiny tail exps
                BASE = [0, 14, 30, 44]
                for s in range(64):
                    for h in range(HEADS_PER_CORE):
                        t = s - BASE[h] - 2
                        if 0 <= t < LT:
                            normalize(h, t)
                    for h in (3, 2, 1, 0):
                        m = s - BASE[h] - 1
                        if 0 <= m < 8:
                            scores2(h, m)
                    for h in (3, 2, 1, 0):  # newest head first: big exp first
                        m = s - BASE[h]
                        if 0 <= m < LT:
                            scores(h, m)
                    if s <= 11:
                        v_chain(psX, s, tag="px")
                    elif s == 12:
                        v_chain(psX, 12, tag="px")
                        v_chain(psX, 13, tag="px")
                    elif s == 13:
                        v_chain(psX, 14, tag="px")
                        v_chain(psX, 15, tag="px")
                    if 16 <= s <= 22 and s % 2 == 0:
                        # q j=1 chains, spread across head 1
                        qk_chain(psX, wq_sb, qTr, 1, (s - 16) // 2,
                                 tag="px", evac="pool")
                    if s == 24:
                        # first k j=1 chain, just before head 2 needs it
                        qk_chain(psX, wk_sb, kTr, 1, 0,
                                 tag="px", evac="pool")
                    if s in (31, 35, 39):
                        # remaining k j=1 chains, just-in-time: chain g is
                        # first read by scores(2, 4g) at step 30+4g
                        qk_chain(psX, wk_sb, kTr, 1, 1 + (s - 31) // 4,
                                 tag="px", evac="pool")
                    for h in range(HEADS_PER_CORE):
                        t = s - BASE[h] - 1
                        if 0 <= t < LT:
                            pv_chain(h, t)
                    for h in range(HEADS_PER_CORE):
                        t = s - BASE[h] - 3
                        if 0 <= t < LT:
                            transpose(h, t)
                    if 49 <= s < 49 + LT:
                        out_proj(s - 49)

    _split_waits(nc)
    return nc


def _build_runner(nc):
    """Build the sharded PJRT executable once (mirrors
    bass2jax.run_bass_via_pjrt) and return a callable in_maps -> results."""
    import jax
    import numpy as _np
    from jax.sharding import Mesh, PartitionSpec
    from jax.experimental.shard_map import shard_map
    from concourse import bass2jax, mybir

    bass2jax.install_neuronx_cc_hook()
    partition_name = (
        nc.partition_id_tensor.name if nc.partition_id_tensor else None
    )
    in_names, out_names, out_avals, zero_outs = [], [], [], []
    for alloc in nc.m.functions[0].allocations:
        if not isinstance(alloc, mybir.MemoryLocationSet):
            continue
        name = alloc.memorylocations[0].name
        if alloc.kind == "ExternalInput":
            if name != partition_name:
                in_names.append(name)
        elif alloc.kind == "ExternalOutput":
            out_names.append(name)
            shape = tuple(alloc.tensor_shape)
            dtype = mybir.dt.np(alloc.dtype)
            out_avals.append(jax.core.ShapedArray(shape, dtype))
            zero_outs.append(_np.zeros(shape, dtype))
    n_params = len(in_names)
    n_outs = len(out_names)
    all_in_names = list(in_names) + list(out_names)
    if partition_name is not None:
        all_in_names.append(partition_name)
    donate = tuple(range(n_params, n_params + n_outs))

    def _body(*args):
        operands = list(args)
        if partition_name is not None:
            operands.append(bass2jax.partition_id_tensor())
        outs = bass2jax._bass_exec_p.bind(
            *operands,
            out_avals=tuple(out_avals),
            in_names=tuple(all_in_names),
            out_names=tuple(out_names),
            lowering_input_output_aliases=(),
            sim_require_finite=True,
            sim_require_nnan=True,
            nc=nc,
        )
        return tuple(outs)

    devices = jax.devices()[:N_CORES]
    mesh = Mesh(_np.asarray(devices), ("core",))
    in_specs = (PartitionSpec("core"),) * (n_params + n_outs)
    out_specs = (PartitionSpec("core"),) * n_outs
    sharded = jax.jit(
        shard_map(
            _body, mesh=mesh, in_specs=in_specs, out_specs=out_specs,
            check_rep=False,
        ),
        donate_argnums=donate,
        keep_unused=True,
    )

    def run(in_maps):
        concat_in = [
            _np.concatenate([_np.asarray(m[nm]) for m in in_maps], axis=0)
            for nm in in_names
        ]
        concat_zeros = [
            _np.zeros((N_CORES * z.shape[0], *z.shape[1:]), z.dtype)
            for z in zero_outs
        ]
        out_arrs = sharded(*concat_in, *concat_zeros)
        return [
            {
                nm: _np.asarray(out_arrs[i]).reshape(
                    N_CORES, *out_avals[i].shape
                )[c]
                for i, nm in enumerate(out_names)
            }
            for c in range(N_CORES)
        ]

    return run


def _numpy_ref(x, attn_mask, Wq, Wk, Wv, Wo):
    xb, Lb, Db = x.shape
    dh = Db // H
    x64 = x.astype(np.float64)
    q = (x64 @ Wq.T.astype(np.float64)).reshape(xb, Lb, H, dh)
    k = (x64 @ Wk.T.astype(np.float64)).reshape(xb, Lb, H, dh)
    v = (x64 @ Wv.T.astype(np.float64)).reshape(xb, Lb, H, dh)
    scores = np.einsum("blhd,bmhd->bhlm", q, k) / np.sqrt(dh)
    scores = np.where(attn_mask[None, None, :, :] == 0, -np.inf, scores)
    scores -= scores.max(axis=-1, keepdims=True)
    e = np.exp(scores)
    attn = e / e.sum(axis=-1, keepdims=True)
    out = np.einsum("bhlm,bmhd->blhd", attn, v).reshape(xb, Lb, Db)
    return (out @ Wo.T.astype(np.float64)).astype(x.dtype)


def _trimask():
    import ml_dtypes

    j = np.arange(128)
    return np.where(j[None, :] >= j[:, None], 1.0, 0.0).astype(
        ml_dtypes.bfloat16
    )


def _make_in_maps(x, Wq, Wk, Wv, Wo):
    import ml_dtypes

    bf = ml_dtypes.bfloat16
    tm = _trimask()
    ident = np.eye(128, dtype=bf)
    xT = [np.ascontiguousarray(x[b].T).astype(bf) for b in range(B)]
    WqT = np.ascontiguousarray(Wq.T).astype(bf)
    WkT = np.ascontiguousarray(Wk.T).astype(bf)
    WvT = np.ascontiguousarray(Wv.T).astype(bf)
    in_maps = []
    for c in range(N_CORES):
        b = c // 4
        s0 = HD * (c % 4)
        sel = slice(s0, s0 + HD)
        in_maps.append(
            {
                "xT": xT[b],
                "wqT": WqT[:, sel],
                "wkT": WkT[:, sel],
                "wvT": WvT[:, sel],
                "woT": np.ascontiguousarray(Wo[:, sel].T).astype(bf),
                "trimask": tm,
                "ident": ident,
            }
        )
    return in_maps


def kernel(x, attn_mask, Wq, Wk, Wv, Wo):
    x = np.asarray(x)
    attn_mask = np.asarray(attn_mask)
    Wq, Wk, Wv, Wo = (np.asarray(a) for a in (Wq, Wk, Wv, Wo))
    causal = x.shape == (B, L, D) and np.array_equal(
        attn_mask != 0, np.tril(np.ones((L, L), dtype=bool))
    )
    if not causal:
        return _numpy_ref(x, attn_mask, Wq, Wk, Wv, Wo)

    if "run" not in _CACHE:
        _CACHE["run"] = _build_runner(_build_program())
    in_maps = _make_in_maps(x, Wq, Wk, Wv, Wo)
    results = _CACHE["run"](in_maps)
    out = np.zeros((B, L, D), dtype=np.float32)
    for c in range(N_CORES):
        out[c // 4] += results[c]["out"]
    return out


# revision 56
# speedup vs baseline: 1.0224x; 1.0224x over previous
"""Multi-head self-attention (B=2, L=2048, D=1024, H=16, causal) on 8
Trainium2 NeuronCores.

Sharding: tensor-parallel over heads x data-parallel over batch.
Core c (0..7) handles batch b = c//4 and heads 4*(c%4) .. 4*(c%4)+3.
Each core computes partial = (softmax(qk^T/8) @ v_heads) @ Wo[:, cols]^T of
shape [L, D]; the host sums the 4 partials of each batch group.

Per-core kernel design (validated on device at rel_err ~4e-3):
  - inputs stream in as bf16 (halves the serial DMA prefix); DMA lands
    directly in matmul-ready tiles, no staging copies; q/k stay f32r
    after the fp32-accumulated projection
  - the q/k j=0 projection wave is emitted contraction-chunk-major so
    it tracks the chunk-interleaved input DMAs; zero-weight `warm`
    matmuls (start=False adds of 0) keep the PE clock ramp at full
    speed through the DMA-gated stretches
  - scores are computed TRANSPOSED (S^T = k q^T per 128-row key chunk,
    causal columns only); exp runs on ScalarE PSUM->SBUF producing P^T
    in bf16 with the 1/sqrt(dh) scale folded in; the diagonal block is
    masked AFTER exp by a 0/1 lower-tri multiply on the Pool engine
    (SBUF-only: GPSIMD/Pool cannot touch PSUM on trn2)
  - P@V is flipped: each 128-query output tile O[q, d] accumulates over
    key chunks with P^T slices as the stationary operand and the 65-wide
    (64 dims + ones column) bf16 v block as the moving operand - the PE
    streams 65 columns per chunk instead of the full query range
  - the ones column gives per-row softmax sums; normalization is a DVE
    reciprocal + per-partition tensor_scalar multiply (no broadcast
    matmuls, no extra activations)
  - normalized O tiles are transposed on the PE (bf16 identity matmul)
    into the packed [128, 2, L] layout the output projection consumes
    with full K=128 contraction chunks
  - one flat software-pipelined loop runs all 4 heads' (scores -> pv ->
    normalize -> transpose) stages at fixed lags; the v projection rides
    head 0, the j=1 q/k chains ride heads 1-2 just-in-time, and the
    output projection + bf16 store DMAs ride head 3, so PE and ScalarE
    stay co-busy end to end
"""

import numpy as np

B, L, D, H = 2, 2048, 1024, 16
DH = D // H  # 64
N_CORES = 8
HEADS_PER_CORE = 4
HD = HEADS_PER_CORE * DH  # 256 head dims per core
NK = D // 128  # 8 contraction chunks
LT = L // 128  # 16 L tiles

_CACHE = {}


# ---------------------------------------------------------------------------
# walrus compat: this compiler build accepts at most ONE sync-wait command
# per instruction, while TileContext attaches one wait per producer proc.
# Hoist surplus waits onto same-engine NOPs inserted just before the
# offending instruction (identical AND semantics).
# ---------------------------------------------------------------------------
def _split_waits(nc):
    import bass_rust
    import concourse.mybir as mybir

    for fn in nc.m.functions:
        for bb in fn.blocks:
            insts = list(bb.instructions)
            out = []
            changed = False
            for inst in insts:
                si = inst.sync_info
                waits = list(si.on_wait) if si is not None and si.on_wait else []
                if len(waits) > 1:
                    changed = True
                    for w in waits[:-1]:
                        out.append(
                            mybir.InstNoOp(
                                name=nc.get_next_instruction_name(),
                                engine=inst.engine,
                                bass_nofuse=True,
                                sync_info=bass_rust.SyncInfo(
                                    on_wait=[w], on_update=[]
                                ),
                            )
                        )
                    inst.sync_info = bass_rust.SyncInfo(
                        on_wait=[waits[-1]], on_update=list(si.on_update or [])
                    )
                out.append(inst)
            if changed:
                try:
                    bb.instructions = out
                except Exception:
                    bb.instructions.clear()
                    bb.instructions.extend(out)


def _build_program():
    import concourse.bass as bass
    import concourse.mybir as mybir
    import concourse.tile as tile

    f32 = mybir.dt.float32
    f32r = mybir.dt.float32r
    bf16 = mybir.dt.bfloat16
    AF = mybir.ActivationFunctionType

    nc = bass.Bass("TRN2", target_bir_lowering=False, debug=False)
    xT_d = nc.dram_tensor("xT", [D, L], bf16, kind="ExternalInput")
    wq_d = nc.dram_tensor("wqT", [D, HD], bf16, kind="ExternalInput")
    wk_d = nc.dram_tensor("wkT", [D, HD], bf16, kind="ExternalInput")
    wv_d = nc.dram_tensor("wvT", [D, HD], bf16, kind="ExternalInput")
    wo_d = nc.dram_tensor("woT", [HD, D], bf16, kind="ExternalInput")
    tm_d = nc.dram_tensor("trimask", [128, 128], bf16, kind="ExternalInput")
    id_d = nc.dram_tensor("ident", [128, 128], bf16, kind="ExternalInput")
    out_d = nc.dram_tensor("out", [L, D], bf16, kind="ExternalOutput")

    with tile.TileContext(nc, pool_alloc_mode="queue") as tc:
        with tc.tile_pool(name="persist", bufs=1) as persist:
            qTr = [persist.tile([128, L], f32r, name=f"qTr{j}") for j in range(2)]
            kTr = [persist.tile([128, L], f32r, name=f"kTr{j}") for j in range(2)]
            # v with a ones column per head: [key%128, keychunk, 65*h + u]
            v_sb = persist.tile([128, LT, HEADS_PER_CORE * (DH + 1)], bf16)
            tm_t = persist.tile([128, 128], bf16)
            id_t = persist.tile([128, 128], bf16)
            woTr = persist.tile([128, 2, D], bf16)
            # packed O^T: [:, j, q] partitions 0-63 head 2j, 64-127 head 2j+1
            otP = persist.tile([128, 2, L], bf16)
            xTr = [
                persist.tile([128, L], bf16, name=f"xTr{c}") for c in range(NK)
            ]
            wq_sb = persist.tile([128, NK, HD], bf16)
            wk_sb = persist.tile([128, NK, HD], bf16)
            wv_sb = persist.tile([128, NK, HD], bf16)

            # warm-up fodder: zero stationary operand makes a matmul that
            # adds 0 to any psum region - used to hold the PE clock ramp
            # at full speed through the DMA-gated load phase
            zt = persist.tile([128, 128], bf16)
            nc.gpsimd.memset(zt[:], 0.0)

            # ---- input DMA, one ordered queue, chunk-interleaved so the
            # q/k wave can follow the transfers chunk by chunk
            nc.sync.dma_start(tm_t[:], tm_d[:])
            nc.sync.dma_start(id_t[:], id_d[:])
            wqv = wq_d[:].rearrange("(c p) n -> p c n", p=128)
            wkv = wk_d[:].rearrange("(c p) n -> p c n", p=128)
            wvv = wv_d[:].rearrange("(c p) n -> p c n", p=128)
            for c in range(NK):
                nc.sync.dma_start(wq_sb[:, c, :], wqv[:, c, :])
                nc.sync.dma_start(wk_sb[:, c, :], wkv[:, c, :])
                if c < NK - 1:
                    nc.sync.dma_start(
                        xTr[c][:], xT_d[c * 128 : (c + 1) * 128, :]
                    )
            # the last x chunk lands in four column quarters so the
            # projection chains (which gate the first exps) finish early
            for g in range(4):
                nc.sync.dma_start(
                    xTr[NK - 1][:, g * 512 : (g + 1) * 512],
                    xT_d[(NK - 1) * 128 : NK * 128, g * 512 : (g + 1) * 512],
                )
            nc.sync.dma_start(
                wv_sb[:], wv_d[:].rearrange("(c p) n -> p c n", p=128)
            )
            nc.sync.dma_start(
                woTr[:], wo_d[:].rearrange("(j p) n -> p j n", p=128)
            )

            # ones column of v (memset once over the strided view)
            vview = v_sb[:].rearrange("p t (h u) -> p t h u", u=DH + 1)
            nc.gpsimd.memset(vview[:, :, :, DH], 1.0)

            def warm(ps, n):
                for _ in range(n):
                    nc.tensor.matmul(
                        ps[:, 0:128], zt[:], id_t[:],
                        start=False, stop=False, skip_group_check=True,
                    )

            # ---------------- phase A wave 1: q,k for head pair j=0 ------
            # (j=1 and v chains are interleaved into the attention loops)
            with tc.tile_pool(name="psA", bufs=8, space="PSUM") as psA:
                wave = []
                for g in range(4):
                    for wt_sb, dst in ((wq_sb, qTr), (wk_sb, kTr)):
                        ps = psA.tile([128, 512], f32, tag="pa",
                                      name=f"pa{len(wave)}")
                        wave.append((ps, wt_sb, dst, g))
                warm(wave[0][0], 8)
                for c in range(NK):
                    for ps, wt_sb, dst, g in wave:
                        nc.tensor.matmul(
                            ps[:],
                            wt_sb[:, c, 0:128],
                            xTr[c][:, g * 512 : (g + 1) * 512],
                            start=(c == 0),
                            stop=(c == NK - 1),
                        )
                    if c < NK - 1:
                        warm(wave[0][0], 2)
                for i, (ps, wt_sb, dst, g) in enumerate(wave):
                    eng = nc.scalar.copy if i % 4 < 2 else nc.vector.tensor_copy
                    eng(dst[0][:, g * 512 : (g + 1) * 512], ps[:])

            # ------------- phase B: attention, software-pipelined -------
            with (
                tc.tile_pool(name="ptp", bufs=2) as ptp,
                tc.tile_pool(name="onp", bufs=3) as onp,
                tc.tile_pool(name="rcp", bufs=3) as rcp,
                tc.tile_pool(name="stg", bufs=3) as stg,
                tc.tile_pool(name="psS", bufs=2, space="PSUM") as psS,
                tc.tile_pool(name="psO", bufs=2, space="PSUM") as psO,
                tc.tile_pool(name="psT", bufs=1, space="PSUM") as psT,
                tc.tile_pool(name="psX", bufs=1, space="PSUM") as psX,
            ):
                PT = {}
                o_ps = {}
                o_nrm = {}

                def hparams(h):
                    return h // 2, 64 * (h % 2)

                def qk_chain(pool, wt_sb, dst, j, g, tag, evac):
                    ps = pool.tile([128, 512], f32, tag=tag, name="qkc")
                    for c in range(NK):
                        nc.tensor.matmul(
                            ps[:],
                            wt_sb[:, c, j * 128 : (j + 1) * 128],
                            xTr[c][:, g * 512 : (g + 1) * 512],
                            start=(c == 0),
                            stop=(c == NK - 1),
                        )
                    if evac == "scalar":
                        nc.scalar.copy(dst[j][:, g * 512 : (g + 1) * 512], ps[:])
                    else:
                        nc.vector.tensor_copy(
                            dst[j][:, g * 512 : (g + 1) * 512], ps[:]
                        )

                def v_chain(pool, t, tag):
                    ps = pool.tile([128, 512], f32, tag=tag, name="vc")
                    for c in range(NK):
                        nc.tensor.matmul(
                            ps[:, 0:HD],
                            xTr[c][:, t * 128 : (t + 1) * 128],
                            wv_sb[:, c, :],
                            start=(c == 0),
                            stop=(c == NK - 1),
                        )
                    vdst = v_sb[:, t, :].rearrange("p (h u) -> p h u", u=DH + 1)
                    nc.vector.tensor_copy(
                        vdst[:, :, 0:DH],
                        ps[:, 0:HD].rearrange("p (h u) -> p h u", u=DH),
                    )

                def scores(h, m):
                    hp, ho = hparams(h)
                    c0 = 128 * m
                    w = L - c0
                    PT[h, m] = ptp.tile(
                        [128, w], bf16, tag=f"pt{m}", name=f"pt{h}_{m}"
                    )
                    # the overlapped head-tail steps borrow the psX bank so
                    # the next head's first scores own the psS rotation
                    use_px = False
                    sw = min(1024, w)
                    ps = (
                        psX.tile([128, 512], f32, tag="px", name="spx")
                        if use_px
                        else psS.tile([128, 1024], f32, tag="st", name="sps")
                    )
                    for n0 in range(0, sw, 512):
                        nw = min(512, sw - n0)
                        nc.tensor.matmul(
                            ps[:, n0 : n0 + nw],
                            kTr[hp][ho : ho + 64, c0 : c0 + 128],
                            qTr[hp][
                                ho : ho + 64,
                                c0 + n0 : c0 + n0 + nw,
                            ],
                            start=True,
                            stop=True,
                        )
                    nc.scalar.activation(
                        PT[h, m][:, 0:sw],
                        ps[:, 0:sw],
                        AF.Exp,
                        scale=0.125,
                    )
                    # zero the masked (key > q) part of the diagonal
                    # block after exp - off the PE->ACT critical path
                    nc.gpsimd.tensor_mul(
                        PT[h, m][:, 0:128], PT[h, m][:, 0:128], tm_t[:]
                    )

                def scores2(h, m):
                    # far half (cols 1024+) of the score row, one step
                    # later: doubles the psS tile reuse distance. PV chains
                    # never need these columns for their first 8 chunks, so
                    # the pv lag stays 1.
                    hp, ho = hparams(h)
                    c0 = 128 * m
                    w = L - c0
                    if w <= 1024:
                        return
                    sw = w - 1024
                    ps = psS.tile([128, 1024], f32, tag="st", name="sps2")
                    for n0 in range(0, sw, 512):
                        nw = min(512, sw - n0)
                        nc.tensor.matmul(
                            ps[:, n0 : n0 + nw],
                            kTr[hp][ho : ho + 64, c0 : c0 + 128],
                            qTr[hp][
                                ho : ho + 64,
                                c0 + 1024 + n0 : c0 + 1024 + n0 + nw,
                            ],
                            start=True,
                            stop=True,
                        )
                    nc.scalar.activation(
                        PT[h, m][:, 1024 : 1024 + sw],
                        ps[:, 0:sw],
                        AF.Exp,
                        scale=0.125,
                    )

                def pv_chain(h, t):
                    o_ps[h, t] = psO.tile(
                        [128, DH + 1], f32, tag="o", name=f"ops{h}_{t}"
                    )
                    for mm in range(t + 1):
                        nc.tensor.matmul(
                            o_ps[h, t][:],
                            PT[h, mm][:, (t - mm) * 128 : (t - mm) * 128 + 128],
                            v_sb[:, mm, h * (DH + 1) : (h + 1) * (DH + 1)],
                            start=(mm == 0),
                            stop=(mm == t),
                        )

                def normalize(h, t):
                    r = rcp.tile([128, 1], f32, tag="r", name="rc")
                    nc.vector.reciprocal(r[:], o_ps[h, t][:, DH : DH + 1])
                    o_nrm[h, t] = onp.tile(
                        [128, DH], bf16, tag="on", name=f"onrm{h}_{t}"
                    )
                    nc.vector.tensor_scalar_mul(
                        o_nrm[h, t][:], o_ps[h, t][:, 0:DH], r[:]
                    )
                    del o_ps[h, t]

                def transpose(h, t):
                    hp, ho = hparams(h)
                    tp = psT.tile([64, 128], bf16, tag="tp", name="tpp")
                    nc.tensor.transpose(tp[:], o_nrm[h, t][:], id_t[:])
                    nc.vector.tensor_copy(
                        otP[ho : ho + 64, hp, t * 128 : (t + 1) * 128],
                        tp[:],
                    )
                    del o_nrm[h, t]

                def out_proj(t):
                    stage = stg.tile([128, D], bf16, tag="ostage", name="stt")
                    ps0 = psX.tile([128, 512], f32, tag="px", name="cpx")
                    ps1 = psS.tile([128, 1024], f32, tag="st", name="cps")
                    pss = [ps0, ps1[:, 0:512]]
                    for n2 in range(2):
                        for j in range(2):
                            nc.tensor.matmul(
                                pss[n2],
                                otP[:, j, t * 128 : (t + 1) * 128],
                                woTr[:, j, n2 * 512 : (n2 + 1) * 512],
                                start=(j == 0),
                                stop=(j == 1),
                            )
                        if t >= 11 or (n2 == 1 and t >= 3):
                            nc.scalar.copy(
                                stage[:, n2 * 512 : (n2 + 1) * 512], pss[n2]
                            )
                        else:
                            nc.vector.tensor_copy(
                                stage[:, n2 * 512 : (n2 + 1) * 512], pss[n2]
                            )
                        nc.sync.dma_start(
                            out_d[t * 128 : (t + 1) * 128,
                                  n2 * 512 : (n2 + 1) * 512],
                            stage[:, n2 * 512 : (n2 + 1) * 512],
                        )

                # head h's 16 scores steps start at BASE[h]; heads 1 and
                # 3 begin two steps before the previous head's scores end,
                # pairing their big early exps with the tiny tail exps
                BASE = [0, 14, 30, 44]
                for s in range(64):
                    for h in range(HEADS_PER_CORE):
                        t = s - BASE[h] - 2
                        if 0 <= t < LT:
                            normalize(h, t)
                    for h in (3, 2, 1, 0):
                        m = s - BASE[h] - 1
                        if 0 <= m < 8:
                            scores2(h, m)
                    for h in (3, 2, 1, 0):  # newest head first: big exp first
                        m = s - BASE[h]
                        if 0 <= m < LT:
                            scores(h, m)
                    if s < LT:
                        v_chain(psX, s, tag="px")
                    if 16 <= s <= 22 and s % 2 == 0:
                        # q j=1 chains, spread across head 1
                        qk_chain(psX, wq_sb, qTr, 1, (s - 16) // 2,
                                 tag="px", evac="pool")
                    if s == 24:
                        # first k j=1 chain, just before head 2 needs it
                        qk_chain(psX, wk_sb, kTr, 1, 0,
                                 tag="px", evac="pool")
                    if s in (31, 35, 39):
                        # remaining k j=1 chains, just-in-time: chain g is
                        # first read by scores(2, 4g) at step 30+4g
                        qk_chain(psX, wk_sb, kTr, 1, 1 + (s - 31) // 4,
                                 tag="px", evac="pool")
                    for h in range(HEADS_PER_CORE):
                        t = s - BASE[h] - 1
                        if 0 <= t < LT:
                            pv_chain(h, t)
                    for h in range(HEADS_PER_CORE):
                        t = s - BASE[h] - 3
                        if 0 <= t < LT:
                            transpose(h, t)
                    if 50 <= s < 50 + LT:
                        out_proj(s - 50)

    _split_waits(nc)
    return nc


def _build_runner(nc):
    """Build the sharded PJRT executable once (mirrors
    bass2jax.run_bass_via_pjrt) and return a callable in_maps -> results."""
    import jax
    import numpy as _np
    from jax.sharding import Mesh, PartitionSpec
    from jax.experimental.shard_map import shard_map
    from concourse import bass2jax, mybir

    bass2jax.install_neuronx_cc_hook()
    partition_name = (
        nc.partition_id_tensor.name if nc.partition_id_tensor else None
    )
    in_names, out_names, out_avals, zero_outs = [], [], [], []
    for alloc in nc.m.functions[0].allocations:
        if not isinstance(alloc, mybir.MemoryLocationSet):
            continue
        name = alloc.memorylocations[0].name
        if alloc.kind == "ExternalInput":
            if name != partition_name:
                in_names.append(name)
        elif alloc.kind == "ExternalOutput":
            out_names.append(name)
            shape = tuple(alloc.tensor_shape)
            dtype = mybir.dt.np(alloc.dtype)
            out_avals.append(jax.core.ShapedArray(shape, dtype))
            zero_outs.append(_np.zeros(shape, dtype))
    n_params = len(in_names)
    n_outs = len(out_names)
    all_in_names = list(in_names) + list(out_names)
    if partition_name is not None:
        all_in_names.append(partition_name)
    donate = tuple(range(n_params, n_params + n_outs))

    def _body(*args):
        operands = list(args)
        if partition_name is not None:
            operands.append(bass2jax.partition_id_tensor())
        outs = bass2jax._bass_exec_p.bind(
            *operands,
            out_avals=tuple(out_avals),
            in_names=tuple(all_in_names),
            out_names=tuple(out_names),
            lowering_input_output_aliases=(),
            sim_require_finite=True,
            sim_require_nnan=True,
            nc=nc,
        )
        return tuple(outs)

    devices = jax.devices()[:N_CORES]
    mesh = Mesh(_np.asarray(devices), ("core",))
    in_specs = (PartitionSpec("core"),) * (n_params + n_outs)
    out_specs = (PartitionSpec("core"),) * n_outs
    sharded = jax.jit(
        shard_map(
            _body, mesh=mesh, in_specs=in_specs, out_specs=out_specs,
            check_rep=False,
        ),
        donate_argnums=donate,
        keep_unused=True,
    )

    def run(in_maps):
        concat_in = [
            _np.concatenate([_np.asarray(m[nm]) for m in in_maps], axis=0)
            for nm in in_names
        ]
        concat_zeros = [
            _np.zeros((N_CORES * z.shape[0], *z.shape[1:]), z.dtype)
            for z in zero_outs
        ]
        out_arrs = sharded(*concat_in, *concat_zeros)
        return [
            {
                nm: _np.asarray(out_arrs[i]).reshape(
                    N_CORES, *out_avals[i].shape
                )[c]
                for i, nm in enumerate(out_names)
            }
            for c in range(N_CORES)
        ]

    return run


def _numpy_ref(x, attn_mask, Wq, Wk, Wv, Wo):
    xb, Lb, Db = x.shape
    dh = Db // H
    x64 = x.astype(np.float64)
    q = (x64 @ Wq.T.astype(np.float64)).reshape(xb, Lb, H, dh)
    k = (x64 @ Wk.T.astype(np.float64)).reshape(xb, Lb, H, dh)
    v = (x64 @ Wv.T.astype(np.float64)).reshape(xb, Lb, H, dh)
    scores = np.einsum("blhd,bmhd->bhlm", q, k) / np.sqrt(dh)
    scores = np.where(attn_mask[None, None, :, :] == 0, -np.inf, scores)
    scores -= scores.max(axis=-1, keepdims=True)
    e = np.exp(scores)
    attn = e / e.sum(axis=-1, keepdims=True)
    out = np.einsum("bhlm,bmhd->blhd", attn, v).reshape(xb, Lb, Db)
    return (out @ Wo.T.astype(np.float64)).astype(x.dtype)


def _trimask():
    import ml_dtypes

    j = np.arange(128)
    return np.where(j[None, :] >= j[:, None], 1.0, 0.0).astype(
        ml_dtypes.bfloat16
    )


def _make_in_maps(x, Wq, Wk, Wv, Wo):
    import ml_dtypes

    bf = ml_dtypes.bfloat16
    tm = _trimask()
    ident = np.eye(128, dtype=bf)
    xT = [np.ascontiguousarray(x[b].T).astype(bf) for b in range(B)]
    WqT = np.ascontiguousarray(Wq.T).astype(bf)
    WkT = np.ascontiguousarray(Wk.T).astype(bf)
    WvT = np.ascontiguousarray(Wv.T).astype(bf)
    in_maps = []
    for c in range(N_CORES):
        b = c // 4
        s0 = HD * (c % 4)
        sel = slice(s0, s0 + HD)
        in_maps.append(
            {
                "xT": xT[b],
                "wqT": WqT[:, sel],
                "wkT": WkT[:, sel],
                "wvT": WvT[:, sel],
                "woT": np.ascontiguousarray(Wo[:, sel].T).astype(bf),
                "trimask": tm,
                "ident": ident,
            }
        )
    return in_maps


def kernel(x, attn_mask, Wq, Wk, Wv, Wo):
    x = np.asarray(x)
    attn_mask = np.asarray(attn_mask)
    Wq, Wk, Wv, Wo = (np.asarray(a) for a in (Wq, Wk, Wv, Wo))
    causal = x.shape == (B, L, D) and np.array_equal(
        attn_mask != 0, np.tril(np.ones((L, L), dtype=bool))
    )
    if not causal:
        return _numpy_ref(x, attn_mask, Wq, Wk, Wv, Wo)

    if "run" not in _CACHE:
        _CACHE["run"] = _build_runner(_build_program())
    in_maps = _make_in_maps(x, Wq, Wk, Wv, Wo)
    results = _CACHE["run"](in_maps)
    out = np.zeros((B, L, D), dtype=np.float32)
    for c in range(N_CORES):
        out[c // 4] += results[c]["out"]
    return out
